# revision 1
# baseline (speedup 1.0000x reference)
"""CrossModalAttention Trainium2 kernel.

Data-parallel over B*T = 32 frames -> 4 frames per core on 8 cores.
Per frame (S=512, D=512, H=8, hd=64):
  Q^T = Wq'.T @ qs^T  (Wq' = Wq * modw[h]/sqrt(hd) folded per head block)
  K^T = Wk.T  @ ks^T
  V   = vs @ Wv + bv          (natural [k, d] layout, bias via rank-1 matmul)
  S_h = Q_h K_h^T             (per head, [s, k] in PSUM)
  p   = sigmoid(5*(S - rowmax(S)))   (temporal_sync cancels inside the
                                      max-subtracted sigmoid, so it is
                                      mathematically a no-op)
  attn = p / (rowsum(p) + 1e-8)      (rowsum fused into the sigmoid op)
  A^T_h = (V_h as lhsT).T @ attn^T   (attn^T via one batched DMA xbar
                                      transpose per score tile)
  out  = A @ Wo + bo          (bias via rank-1 matmul, f32 out)

All matmuls bf16 x bf16 -> f32 PSUM.  Head pairs (2a, 2a+1) sit in
partition halves 0:64 / 64:128 of the same d-tile, so their score
matmuls (K=64, row groups 0-1 vs 2-3) and attend matmuls (M=64, col
groups via tile_position) are interleaved to run concurrently on the
PE's 32x32 sub-arrays.
"""

import math

import numpy as np
import ml_dtypes

import concourse.bass as bass
import concourse.bacc as bacc
import concourse.mybir as mybir
import concourse.tile as tile
from concourse import bass_utils

BF16 = mybir.dt.bfloat16
F32 = mybir.dt.float32
AF = mybir.ActivationFunctionType

B, T, S, D = 2, 16, 512, 512
H, HD = 8, 64
NCORES = 8
FRAMES = B * T // NCORES  # 4 frames per core
NT = D // 128  # 4 tiles of 128 along any 512 dim


def _emit(tc, nc, aps):
    qs, ks, vs, wq, wk, wv, wo, bq, bk, bv, bo, out = aps

    with tc.tile_pool(name="wpool", bufs=1) as wpool, \
         tc.tile_pool(name="npool", bufs=2) as npool, \
         tc.tile_pool(name="tpool", bufs=2) as tpool, \
         tc.tile_pool(name="attnpool", bufs=3) as attnpool, \
         tc.tile_pool(name="atpool", bufs=2) as atpool, \
         tc.tile_pool(name="outpool", bufs=2) as outpool, \
         tc.tile_pool(name="mmps", bufs=2, space="PSUM") as mmps, \
         tc.tile_pool(name="sps", bufs=4, space="PSUM") as sps, \
         tc.tile_pool(name="aps_pool", bufs=2, space="PSUM") as aps_pool:

        # ---------- per-frame stage emitters (two-frame pipeline) ----------
        def alloc_state(f):
            st = {}
            for nm in ("qn", "kn", "vn", "qt", "kt", "vt"):
                pool = npool if nm[1] == "n" else tpool
                st[nm] = pool.tile([128, NT, 512], BF16, tag=nm,
                                   name=f"{nm}_{f}")
            st["qT"] = tpool.tile([128, NT, 512], BF16, tag="qT", name=f"qT_{f}")
            st["kT"] = tpool.tile([128, NT, 512], BF16, tag="kT", name=f"kT_{f}")
            st["vN"] = tpool.tile([128, NT, 512], BF16, tag="vN", name=f"vN_{f}")
            return st

        SRC = {"qn": qs, "kn": ks, "vn": vs}

        def emit_load1(f, st, dn):
            # split each cast in half so transposes can start sooner
            src = SRC[dn][f].rearrange("(a p) d -> p a d", p=128)
            nc.gpsimd.dma_start(st[dn][:, 0:2, :], src[:, 0:2, :])
            nc.gpsimd.dma_start(st[dn][:, 2:4, :], src[:, 2:4, :])

        def emit_transposes1(st, tn, ti):
            nn = tn[0] + "n"
            for i in range(NT):
                eng = nc.sync if (ti * NT + i) % 2 == 0 else nc.scalar
                eng.dma_start(
                    st[tn][:, :, 128 * i:128 * i + 128],
                    st[nn][:, i, :], transpose=True)

        def emit_load(f, st):
            for dn in ("qn", "kn", "vn"):
                emit_load1(f, st, dn)

        def emit_transposes(st):
            for ti, tn in enumerate(("qt", "kt", "vt")):
                emit_transposes1(st, tn, ti)

        def emit_proj_qk(st, which):
            dst, w_sb, src, b_sb = (
                (st["qT"], wq_sb, st["qt"], bq_sb) if which == "q"
                else (st["kT"], wk_sb, st["kt"], bk_sb))
            for j in range(NT):
                ps = mmps.tile([128, 512], F32, tag="mm", name=f"mmq_{j}")
                for i in range(NT):
                    nc.tensor.matmul(
                        ps[:], w_sb[:, i, 128 * j:128 * j + 128],
                        src[:, i, :], start=(i == 0), stop=(i == NT - 1))
                nc.vector.tensor_scalar_add(dst[:, j, :], ps[:], b_sb[:, j:j + 1])

        def emit_proj_v(st):
            for m in range(NT):
                ps = mmps.tile([128, 512], F32, tag="mm", name=f"mmv_{m}")
                nc.tensor.matmul(ps[:], ones_sb[0:1, :], bv_sb[0:1, :],
                                 start=True, stop=False)
                for i in range(NT):
                    nc.tensor.matmul(
                        ps[:], st["vt"][:, i, 128 * m:128 * m + 128],
                        wv_sb[:, i, :], start=False, stop=(i == NT - 1))
                nc.scalar.copy(st["vN"][:, m, :], ps[:])

        # ---------- attention stages ----------
        def stage_a(s_ps):
            mx = attnpool.tile([128, 1], F32, tag="mx", bufs=4)
            nm5 = attnpool.tile([128, 1], F32, tag="nm5", bufs=4)
            rs = attnpool.tile([128, 1], F32, tag="rs", bufs=4)
            p_sb = attnpool.tile([128, 512], BF16, tag="p", bufs=6)
            nc.vector.reduce_max(mx[:], s_ps[:], axis=mybir.AxisListType.X)
            nc.vector.tensor_scalar_mul(nm5[:], mx[:], -5.0)
            nc.scalar.activation(p_sb[:], s_ps[:], AF.Sigmoid,
                                 bias=nm5[:], scale=5.0, accum_out=rs[:])
            return rs, p_sb

        def stage_b(rs, p_sb, attnT, si, eng):
            rsi = attnpool.tile([128, 1], F32, tag="rsi", bufs=4)
            nc.vector.reciprocal(rsi[:], rs[:])
            nc.vector.tensor_scalar_mul(p_sb[:], p_sb[:], rsi[:])
            eng.dma_start(attnT[:, :, 128 * si:128 * si + 128],
                          p_sb[:], transpose=True)

        def pair_scores(f, st, a, pend_b):
            attnTs = []
            for h in (2 * a, 2 * a + 1):
                attnTs.append(
                    attnpool.tile([128, NT, 512], BF16, tag="attnT",
                                  name=f"attnT_{f}_{h}", bufs=4))
            for si in range(NT):
                s_list = []
                for idx, h in enumerate((2 * a, 2 * a + 1)):
                    lo = 64 * (h % 2)
                    qh = st["qT"][lo:lo + 64, a, :]
                    kh = st["kT"][lo:lo + 64, a, :]
                    s_ps = sps.tile([128, 512], F32, tag="s",
                                    name=f"s_{f}_{h}_{si}")
                    nc.tensor.matmul(s_ps[:], qh[:, 128 * si:128 * si + 128],
                                     kh, start=True, stop=True)
                    s_list.append((s_ps, idx))
                new_b = []
                for s_ps, idx in s_list:
                    rs, p_sb = stage_a(s_ps)
                    eng = nc.sync if idx == 0 else nc.scalar
                    new_b.append((rs, p_sb, attnTs[idx], si, eng))
                while pend_b:
                    stage_b(*pend_b.pop(0))
                pend_b.extend(new_b)
            return attnTs

        def pair_attend(f, st, aT, a, attnTs):
            a_ps = aps_pool.tile([128, 512], F32, tag="a", name=f"a_ps_{f}_{a}")
            for idx, h in enumerate((2 * a, 2 * a + 1)):
                lo = 64 * (h % 2)
                for ki in range(NT):
                    nc.tensor.matmul(
                        a_ps[lo:lo + 64, :],
                        st["vN"][:, ki, 64 * h:64 * h + 64],
                        attnTs[idx][:, ki, :],
                        start=(ki == 0), stop=(ki == NT - 1),
                        tile_position=(0, lo))
            nc.vector.tensor_copy(aT[:, a, :], a_ps[:])

        def emit_outproj(f, st, aT):
            outsb = outpool.tile([128, NT, 512], F32, tag="outsb",
                                 name=f"outsb_{f}")
            for stp in range(NT):
                ps = mmps.tile([128, 512], F32, tag="mm", name=f"mmo_{stp}")
                nc.tensor.matmul(ps[:], ones_sb[0:1, :], bo_sb[0:1, :],
                                 start=True, stop=False)
                for j in range(NT):
                    nc.tensor.matmul(
                        ps[:], aT[:, j, 128 * stp:128 * stp + 128],
                        wo_sb[:, j, :], start=False, stop=(j == NT - 1))
                nc.scalar.copy(outsb[:, stp, :], ps[:])
            nc.gpsimd.dma_start(
                out[f].rearrange("(a p) d -> p a d", p=128), outsb[:])

        # ---------- prologue: frame 0 load first, then transposes,
        # weights just before the projections need them ----------
        st0 = alloc_state(0)
        emit_load(0, st0)
        emit_transposes(st0)

        wq_sb = wpool.tile([128, NT, 512], BF16)
        wk_sb = wpool.tile([128, NT, 512], BF16)
        wv_sb = wpool.tile([128, NT, 512], BF16)
        wo_sb = wpool.tile([128, NT, 512], BF16)
        for w_sb, w_dr in ((wq_sb, wq), (wk_sb, wk), (wv_sb, wv), (wo_sb, wo)):
            nc.sync.dma_start(w_sb[:], w_dr.rearrange("(a p) n -> p a n", p=128))
        bq_sb = wpool.tile([128, NT], F32)
        bk_sb = wpool.tile([128, NT], F32)
        nc.sync.dma_start(bq_sb[:], bq.rearrange("(a p) -> p a", p=128))
        nc.sync.dma_start(bk_sb[:], bk.rearrange("(a p) -> p a", p=128))
        bv_sb = wpool.tile([1, 512], BF16)
        bo_sb = wpool.tile([1, 512], BF16)
        nc.gpsimd.dma_start(bv_sb[:], bv.rearrange("(a n) -> a n", a=1))
        nc.gpsimd.dma_start(bo_sb[:], bo.rearrange("(a n) -> a n", a=1))
        ones_sb = wpool.tile([1, 128], BF16)
        nc.vector.memset(ones_sb[:], 1.0)
        warm = wpool.tile([1, 1], F32)
        nc.scalar.activation(warm[:], ones_sb[0:1, 0:1], AF.Sigmoid)

        emit_proj_qk(st0, "q")
        emit_proj_qk(st0, "k")
        emit_proj_v(st0)

        # ---------- steady state: attention(f) interleaved with
        # load+transpose+proj of frame f+1; transposes lag the casts by
        # two pair-rounds so HWDGE streams never block on them ----------
        st = st0
        for f in range(FRAMES):
            nxt = alloc_state(f + 1) if f + 1 < FRAMES else None
            if nxt:
                fillers = [
                    lambda: emit_load1(f + 1, nxt, "qn"),
                    lambda: (emit_load1(f + 1, nxt, "kn"),
                             emit_transposes1(nxt, "qt", 0)),
                    lambda: (emit_load1(f + 1, nxt, "vn"),
                             emit_transposes1(nxt, "kt", 1)),
                    lambda: (emit_transposes1(nxt, "vt", 2),
                             emit_proj_qk(nxt, "q"), emit_proj_qk(nxt, "k"),
                             emit_proj_v(nxt)),
                ]
            else:
                fillers = [lambda: None] * 4
            aT = atpool.tile([128, NT, 512], BF16, tag="aT", name=f"aT_{f}")
            pend_b = []
            pend = None
            for a in range(H // 2):
                attnTs = pair_scores(f, st, a, pend_b)
                fillers[a]()
                if pend is not None:
                    pair_attend(f, st, aT, *pend)
                pend = (a, attnTs)
            while pend_b:
                stage_b(*pend_b.pop(0))
            pair_attend(f, st, aT, *pend)
            emit_outproj(f, st, aT)
            st = nxt


def build_nc():
    nc = bacc.Bacc("TRN2", target_bir_lowering=False, debug=False,
                   num_devices=NCORES)
    qs = nc.dram_tensor("qs", (FRAMES, S, D), F32, kind="ExternalInput").ap()
    ks = nc.dram_tensor("ks", (FRAMES, S, D), F32, kind="ExternalInput").ap()
    vs = nc.dram_tensor("vs", (FRAMES, S, D), F32, kind="ExternalInput").ap()
    wq = nc.dram_tensor("wq", (D, D), BF16, kind="ExternalInput").ap()
    wk = nc.dram_tensor("wk", (D, D), BF16, kind="ExternalInput").ap()
    wv = nc.dram_tensor("wv", (D, D), BF16, kind="ExternalInput").ap()
    wo = nc.dram_tensor("wo", (D, D), BF16, kind="ExternalInput").ap()
    bq = nc.dram_tensor("bq", (D,), F32, kind="ExternalInput").ap()
    bk = nc.dram_tensor("bk", (D,), F32, kind="ExternalInput").ap()
    bv = nc.dram_tensor("bv", (D,), BF16, kind="ExternalInput").ap()
    bo = nc.dram_tensor("bo", (D,), BF16, kind="ExternalInput").ap()
    out = nc.dram_tensor("out", (FRAMES, S, D), F32, kind="ExternalOutput").ap()
    with tile.TileContext(nc) as tc:
        _emit(tc, nc, (qs, ks, vs, wq, wk, wv, wo, bq, bk, bv, bo, out))
    nc.compile()
    return nc


_NC = None


def _get_nc():
    global _NC
    if _NC is None:
        _NC = build_nc()
    return _NC


def make_in_maps(query_spikes, key_spikes, value_spikes, Wq, bq, Wk, bk,
                 Wv, bv, Wo, bo, modality_weights, temporal_sync,
                 query_modality, key_modality):
    qm = int(query_modality)
    km = int(key_modality)
    mw = np.asarray(modality_weights, np.float32)
    c = (mw[qm] * mw[km]) / np.float32(math.sqrt(HD))  # [H]
    # fold per-head score scale into the Q projection
    scale_cols = np.repeat(c, HD)  # [D]
    wq_s = (np.asarray(Wq, np.float32) * scale_cols[None, :])
    bq_s = (np.asarray(bq, np.float32) * scale_cols)

    bf = lambda a: np.asarray(a, np.float32).astype(ml_dtypes.bfloat16)
    shared = {
        "wq": bf(wq_s), "wk": bf(Wk), "wv": bf(Wv), "wo": bf(Wo),
        "bq": np.asarray(bq_s, np.float32), "bk": np.asarray(bk, np.float32),
        "bv": bf(bv), "bo": bf(bo),
    }
    qs_all = np.asarray(query_spikes, np.float32).reshape(B * T, S, D)
    ks_all = np.asarray(key_spikes, np.float32).reshape(B * T, S, D)
    vs_all = np.asarray(value_spikes, np.float32).reshape(B * T, S, D)
    in_maps = []
    for core in range(NCORES):
        sl = slice(core * FRAMES, (core + 1) * FRAMES)
        in_maps.append({
            "qs": np.ascontiguousarray(qs_all[sl]),
            "ks": np.ascontiguousarray(ks_all[sl]),
            "vs": np.ascontiguousarray(vs_all[sl]),
            **shared,
        })
    return in_maps


def kernel(**inputs):
    nc = _get_nc()
    in_maps = make_in_maps(**inputs)
    res = bass_utils.run_bass_kernel_spmd(
        nc, in_maps, core_ids=list(range(NCORES)))
    out = np.concatenate([np.asarray(r["out"], np.float32)
                          for r in res.results], axis=0)
    return out.reshape(B, T, S, D)



# revision 3
# speedup vs baseline: 1.2281x; 1.2281x over previous
"""CrossModalAttention Trainium2 kernel, v3.

Data-parallel over B*T = 32 frames -> 4 frames per core on 8 cores.
fp16 on-chip; f32 PSUM/stats.  Host-side algebra (see v2): bk drops, bq
folds into the Q projection evac bias, -5*mw/sqrt(hd) folds into Wq, the
sigmoid bias is the DVE rowmin, bv/bo become a constant row added in
numpy.  The DMA-transpose/DMA serialization in the scheduler makes DMA
op COUNT the scarce resource, so v3 uses one packed qkv load, ONE
whole-frame input transpose, TWO half-frame attn transposes and one
store per frame (5 DMA ops total).
"""

import math

import numpy as np

import concourse.bass as bass
import concourse.bacc as bacc
import concourse.mybir as mybir
import concourse.tile as tile
from concourse import bass_utils

F16 = mybir.dt.float16
F32 = mybir.dt.float32
AF = mybir.ActivationFunctionType
ALU = mybir.AluOpType

B, T, S, D = 2, 16, 512, 512
H, HD = 8, 64
NCORES = 8
FRAMES = B * T // NCORES  # 4 frames per core
NT = D // 128


def _emit(tc, nc, aps):
    qkv, wall, bq5, out = aps

    with tc.tile_pool(name="wpool", bufs=1) as wpool, \
         tc.tile_pool(name="inpool", bufs=2) as inpool, \
         tc.tile_pool(name="tpool", bufs=2) as tpool, \
         tc.tile_pool(name="projpool", bufs=2) as projpool, \
         tc.tile_pool(name="ppool", bufs=1) as ppool, \
         tc.tile_pool(name="apool", bufs=2) as apool, \
         tc.tile_pool(name="statpool", bufs=8) as statpool, \
         tc.tile_pool(name="sps", bufs=2, space="PSUM") as sps, \
         tc.tile_pool(name="attps", bufs=2, space="PSUM") as attps, \
         tc.tile_pool(name="mmps", bufs=2, space="PSUM") as mmps:

        # ---------------- weights (one-time, one packed DMA) -------------
        walls = wpool.tile([128, 4, NT, 512], F16)  # w, i, n
        bq5_sb = wpool.tile([128, NT], F32)
        wq_sb = walls[:, 0, :, :]
        wk_sb = walls[:, 1, :, :]
        wv_sb = walls[:, 2, :, :]
        wo_sb = walls[:, 3, :, :]

        def load_weights():
            nc.gpsimd.dma_start(walls[:], wall[:])
            nc.gpsimd.dma_start(bq5_sb[:],
                                bq5.rearrange("(i p) -> p i", p=128))

        # ---------------- per-frame state ----------------
        def alloc_state(f):
            st = {}
            # natural: [s%128, stile, tensor, d] -- one contiguous region
            st["qkv"] = inpool.tile([128, NT, 3, 512], F16, tag="qkv",
                                    name=f"qkv_{f}")
            # transposed: [d%128, stile, tensor, dblk, s%128]
            st["qkvT"] = tpool.tile([128, NT, 3, NT, 128], F16, tag="qkvT",
                                    name=f"qkvT_{f}")
            # projections: [dout%128 (head pair-stacked), pair, s]
            st["qT"] = projpool.tile([128, NT, 512], F16, tag="qT",
                                     name=f"qT_{f}")
            st["kT"] = projpool.tile([128, NT, 512], F16, tag="kT",
                                     name=f"kT_{f}")
            # V natural: [k%128, kblk, dout]
            st["vN"] = projpool.tile([128, NT, 512], F16, tag="vN", bufs=3,
                                     name=f"vN_{f}")
            return st

        def emit_load(f, st):
            nc.gpsimd.dma_start(st["qkv"][:], qkv[f])

        def emit_transpose_in(st):
            nc.sync.dma_start(st["qkvT"][:], st["qkv"][:], transpose=True)

        def _rhs(st, t, i):
            # [din%128, (stile), s%128] strided view == [din, s] for chunk i
            return st["qkvT"][:, :, t, i, :]

        def emit_proj_qk(st):
            for j in range(NT):
                ps = mmps.tile([128, 512], F32, tag="mm", name=f"mmq_{j}")
                for i in range(NT):
                    nc.tensor.matmul(
                        ps[:], wq_sb[:, i, 128 * j:128 * j + 128],
                        _rhs(st, 0, i), start=(i == 0), stop=(i == NT - 1))
                nc.scalar.activation(st["qT"][:, j, :], ps[:],
                                     AF.Identity, bias=bq5_sb[:, j:j + 1])
            for j in range(NT):
                ps = mmps.tile([128, 512], F32, tag="mm", name=f"mmk_{j}")
                for i in range(NT):
                    nc.tensor.matmul(
                        ps[:], wk_sb[:, i, 128 * j:128 * j + 128],
                        _rhs(st, 1, i), start=(i == 0), stop=(i == NT - 1))
                nc.vector.tensor_copy(st["kT"][:, j, :], ps[:])

        def emit_proj_v(st):
            for m in range(NT):
                ps = mmps.tile([128, 512], F32, tag="mm", name=f"mmv_{m}")
                for i in range(NT):
                    nc.tensor.matmul(
                        ps[:], st["qkvT"][:, m, 2, i, :],
                        wv_sb[:, i, :], start=(i == 0), stop=(i == NT - 1))
                if m % 2 == 0:
                    nc.scalar.activation(st["vN"][:, m, :], ps[:], AF.Copy)
                else:
                    nc.vector.tensor_copy(st["vN"][:, m, :], ps[:])

        # ---------------- attention ----------------
        def emit_scores(f, st, p2, si, a, norm_eng):
            s_ps = sps.tile([128, 2, 512], F32, tag="s",
                            name=f"s_{f}_{si}_{a}")
            for j, h in enumerate((2 * a, 2 * a + 1)):
                lo = 64 * (h % 2)
                nc.tensor.matmul(
                    s_ps[:, j, :],
                    st["qT"][lo:lo + 64, a, 128 * si:128 * si + 128],
                    st["kT"][lo:lo + 64, a, :], start=True, stop=True)
            m5 = statpool.tile([128, 2], F32, tag="m5",
                               name=f"m5_{f}_{si}_{a}")
            rs = statpool.tile([128, 2], F32, tag="rs",
                               name=f"rs_{f}_{si}_{a}")
            nc.vector.tensor_reduce(m5[:], s_ps[:], mybir.AxisListType.X,
                                    ALU.min)
            for j, h in enumerate((2 * a, 2 * a + 1)):
                nc.scalar.activation(p2[:, si % 2, h, :], s_ps[:, j, :],
                                     AF.Sigmoid, bias=m5[:, j:j + 1],
                                     scale=-1.0, accum_out=rs[:, j:j + 1])
            rsi = statpool.tile([128, 2], F32, tag="rsi",
                                name=f"rsi_{f}_{si}_{a}")
            nc.vector.reciprocal(rsi[:], rs[:])
            for j, h in enumerate((2 * a, 2 * a + 1)):
                norm_eng.tensor_scalar(p2[:, si % 2, h, :],
                                       p2[:, si % 2, h, :],
                                       rsi[:, j:j + 1], None, ALU.mult)

        def emit_attend_half(f, st, attnT_h, aT, half):
            """All 4 head pairs for one q-half (256 cols); two pairs share
            one psum bank."""
            for a in range(4):
                a_ps = attps.tile([128, 256], F32, tag="att",
                                  name=f"aps_{f}_{half}_{a}")
                for h in (2 * a, 2 * a + 1):
                    lo = 64 * (h % 2)
                    for kb in range(NT):
                        nc.tensor.matmul(
                            a_ps[lo:lo + 64, :],
                            st["vN"][:, kb, 64 * h:64 * h + 64],
                            attnT_h[:, :, 4 * h + kb, :],
                            start=(kb == 0), stop=(kb == NT - 1),
                            tile_position=(0, lo))
                nc.vector.tensor_copy(
                    aT[:, a, 256 * half:256 * half + 256], a_ps[:])

        def emit_outproj(f, aT, outsb, stp):
            ps = mmps.tile([128, 512], F32, tag="mm", name=f"mmo_{f}_{stp}")
            for j in range(NT):
                nc.tensor.matmul(
                    ps[:], aT[:, j, 128 * stp:128 * stp + 128],
                    wo_sb[:, j, :], start=(j == 0), stop=(j == NT - 1))
            if stp % 2 == 0:
                nc.scalar.activation(outsb[:, stp, :], ps[:], AF.Copy)
            else:
                nc.vector.tensor_copy(outsb[:, stp, :], ps[:])
            if stp == NT - 1:
                nc.gpsimd.dma_start(
                    out[f].rearrange("(a p) d -> p a d", p=128), outsb[:])

        # ---------------- schedule ----------------
        st0 = alloc_state(0)
        emit_load(0, st0)
        load_weights()
        emit_transpose_in(st0)
        emit_proj_qk(st0)
        emit_proj_v(st0)

        st = st0
        prev = None  # (f, st, attnT_B, aT, outsb) awaiting half-B tail
        for f in range(FRAMES):
            nxt = alloc_state(f + 1) if f + 1 < FRAMES else None
            p2 = [ppool.tile([128, 2, H, 512], F16, tag=f"p{g}",
                             name=f"p_{f}_{g}") for g in range(2)]
            attnTs = [ppool.tile([128, 2, 32, 128], F16, tag=f"attnT{g}",
                                 name=f"attnT_{f}_{g}") for g in range(2)]
            aT = apool.tile([128, NT, 512], F16, tag="aT", name=f"aT_{f}")
            outsb = apool.tile([128, NT, 512], F16, tag="outsb",
                               name=f"outsb_{f}")
            fillers = {0: [], 1: [], 2: [], 3: []}
            if prev is not None:
                fp, stp, attnTBp, aTp, outsbp = prev
                fillers[0].append(
                    lambda: (emit_attend_half(fp, stp, attnTBp, aTp, 1),
                             emit_outproj(fp, aTp, outsbp, 2)))
                fillers[1].append(
                    lambda: emit_outproj(fp, aTp, outsbp, 3))
            if nxt:
                fillers[0].append(lambda: emit_load(f + 1, nxt))
                fillers[1].append(lambda: emit_transpose_in(nxt))
                fillers[2].append(lambda: emit_proj_qk(nxt))
                fillers[3].append(lambda: emit_proj_v(nxt))
            for si in range(NT):
                for a in range(4):
                    on_dve = (si == 0 and a < 2)
                    norm_eng = nc.vector if on_dve else nc.gpsimd
                    emit_scores(f, st, p2[si // 2], si, a, norm_eng)
                if si % 2 == 1:
                    # transpose the completed half (si-1, si)
                    nc.sync.dma_start(attnTs[si // 2][:],
                                      p2[si // 2][:], transpose=True)
                for fn in fillers[si]:
                    fn()
            # half-A attend + its out-proj columns still inside frame f
            emit_attend_half(f, st, attnTs[0], aT, 0)
            emit_outproj(f, aT, outsb, 0)
            emit_outproj(f, aT, outsb, 1)
            prev = (f, st, attnTs[1], aT, outsb)
            st = nxt
        fp, stp, attnTBp, aTp, outsbp = prev
        emit_attend_half(fp, stp, attnTBp, aTp, 1)
        emit_outproj(fp, aTp, outsbp, 2)
        emit_outproj(fp, aTp, outsbp, 3)


def build_nc():
    nc = bacc.Bacc("TRN2", target_bir_lowering=False, debug=False,
                   num_devices=NCORES)
    qkv = nc.dram_tensor("qkv", (FRAMES, 128, NT, 3, D), F32,
                         kind="ExternalInput").ap()
    wall = nc.dram_tensor("wall", (128, 4, NT, D), F16,
                          kind="ExternalInput").ap()
    bq5 = nc.dram_tensor("bq5", (D,), F32, kind="ExternalInput").ap()
    out = nc.dram_tensor("out", (FRAMES, S, D), F16,
                         kind="ExternalOutput").ap()
    with tile.TileContext(nc) as tc:
        _emit(tc, nc, (qkv, wall, bq5, out))
    nc.compile()
    return nc


_NC = None


def _get_nc():
    global _NC
    if _NC is None:
        _NC = build_nc()
    return _NC


def make_in_maps(query_spikes, key_spikes, value_spikes, Wq, bq, Wk, bk,
                 Wv, bv, Wo, bo, modality_weights, temporal_sync,
                 query_modality, key_modality):
    qm = int(query_modality)
    km = int(key_modality)
    mw = np.asarray(modality_weights, np.float32)
    c = (mw[qm] * mw[km]) / np.float32(math.sqrt(HD))  # [H]
    scale_cols = np.repeat(-5.0 * c, HD).astype(np.float32)  # [D]
    wq_s = np.asarray(Wq, np.float32) * scale_cols[None, :]
    bq5 = (np.asarray(bq, np.float32) * scale_cols).astype(np.float32)

    f16 = lambda a: np.asarray(a, np.float32).astype(np.float16)
    # wall[p, w, i, n] = W_w[i*128+p, n]
    wall = np.stack([f16(wq_s), f16(Wk), f16(Wv), f16(Wo)])  # [4, D, D]
    wall = np.ascontiguousarray(
        wall.reshape(4, NT, 128, D).transpose(2, 0, 1, 3))
    shared = {"wall": wall, "bq5": bq5}
    # qkv_all[f, p, a, t, d] = tensor_t[f, a*128+p, d]
    qkv_all = np.stack([
        np.asarray(query_spikes, np.float32).reshape(B * T, S, D),
        np.asarray(key_spikes, np.float32).reshape(B * T, S, D),
        np.asarray(value_spikes, np.float32).reshape(B * T, S, D),
    ], axis=1)  # [B*T, 3, S, D]
    qkv_all = qkv_all.reshape(B * T, 3, NT, 128, D).transpose(0, 3, 2, 1, 4)
    in_maps = []
    for core in range(NCORES):
        sl = slice(core * FRAMES, (core + 1) * FRAMES)
        in_maps.append({
            "qkv": np.ascontiguousarray(qkv_all[sl]),
            **shared,
        })
    return in_maps


def host_bias(Wv_np, bv_np, Wo_np, bo_np):
    return (np.asarray(bv_np, np.float64) @ np.asarray(Wo_np, np.float64)
            + np.asarray(bo_np, np.float64)).astype(np.float32)


def kernel(**inputs):
    nc = _get_nc()
    in_maps = make_in_maps(**inputs)
    res = bass_utils.run_bass_kernel_spmd(
        nc, in_maps, core_ids=list(range(NCORES)))
    out = np.concatenate([np.asarray(r["out"], np.float16).astype(np.float32)
                          for r in res.results], axis=0)
    out += host_bias(inputs["Wv"], inputs["bv"], inputs["Wo"], inputs["bo"])
    return out.reshape(B, T, S, D)


# revision 4
# speedup vs baseline: 1.2807x; 1.0428x over previous
"""CrossModalAttention Trainium2 kernel, v3.

Data-parallel over B*T = 32 frames -> 4 frames per core on 8 cores.
fp16 on-chip; f32 PSUM/stats.  Host-side algebra (see v2): bk drops, bq
folds into the Q projection evac bias, -5*mw/sqrt(hd) folds into Wq, the
sigmoid bias is the DVE rowmin, bv/bo become a constant row added in
numpy.  The DMA-transpose/DMA serialization in the scheduler makes DMA
op COUNT the scarce resource, so v3 uses one packed qkv load, ONE
whole-frame input transpose, TWO half-frame attn transposes and one
store per frame (5 DMA ops total).
"""

import math

import numpy as np

import concourse.bass as bass
import concourse.bacc as bacc
import concourse.mybir as mybir
import concourse.tile as tile
from concourse import bass_utils

F16 = mybir.dt.float16
F32 = mybir.dt.float32
AF = mybir.ActivationFunctionType
ALU = mybir.AluOpType

B, T, S, D = 2, 16, 512, 512
H, HD = 8, 64
NCORES = 8
FRAMES = B * T // NCORES  # 4 frames per core
NT = D // 128


def _emit(tc, nc, aps):
    qkv, wall, bq5, out = aps

    with tc.tile_pool(name="wpool", bufs=1) as wpool, \
         tc.tile_pool(name="inpool", bufs=2) as inpool, \
         tc.tile_pool(name="tpool", bufs=2) as tpool, \
         tc.tile_pool(name="projpool", bufs=2) as projpool, \
         tc.tile_pool(name="ppool", bufs=1) as ppool, \
         tc.tile_pool(name="apool", bufs=2) as apool, \
         tc.tile_pool(name="statpool", bufs=8) as statpool, \
         tc.tile_pool(name="sps", bufs=2, space="PSUM") as sps, \
         tc.tile_pool(name="attps", bufs=2, space="PSUM") as attps, \
         tc.tile_pool(name="mmps", bufs=2, space="PSUM") as mmps:

        # ---------------- weights (one-time, one packed DMA) -------------
        walls = wpool.tile([128, 4, NT, 512], F16)  # w, i, n
        bq5_sb = wpool.tile([128, NT], F32)
        wq_sb = walls[:, 0, :, :]
        wk_sb = walls[:, 1, :, :]
        wv_sb = walls[:, 2, :, :]
        wo_sb = walls[:, 3, :, :]

        def load_weights():
            nc.gpsimd.dma_start(walls[:], wall[:])
            nc.gpsimd.dma_start(bq5_sb[:],
                                bq5.rearrange("(i p) -> p i", p=128))

        # ---------------- per-frame state ----------------
        def alloc_state(f):
            st = {}
            # natural: [s%128, stile, tensor, d] -- one contiguous region
            st["qkv"] = inpool.tile([128, NT, 3, 512], F16, tag="qkv",
                                    name=f"qkv_{f}")
            # transposed: [d%128, stile, tensor, dblk, s%128]
            st["qkvT"] = tpool.tile([128, NT, 3, NT, 128], F16, tag="qkvT",
                                    name=f"qkvT_{f}")
            # projections: [dout%128 (head pair-stacked), pair, s]
            st["qT"] = projpool.tile([128, NT, 512], F16, tag="qT",
                                     name=f"qT_{f}")
            st["kT"] = projpool.tile([128, NT, 512], F16, tag="kT",
                                     name=f"kT_{f}")
            # V natural: [k%128, kblk, dout]
            st["vN"] = projpool.tile([128, NT, 512], F16, tag="vN", bufs=3,
                                     name=f"vN_{f}")
            return st

        def emit_load(f, st, half=None):
            if half is None:
                nc.gpsimd.dma_start(st["qkv"][:], qkv[f])
            else:
                sl = slice(2 * half, 2 * half + 2)
                nc.gpsimd.dma_start(st["qkv"][:, sl, :, :], qkv[f][:, sl, :, :])

        def emit_transpose_in(st, half=None):
            if half is None:
                nc.sync.dma_start(st["qkvT"][:], st["qkv"][:], transpose=True)
            else:
                sl = slice(2 * half, 2 * half + 2)
                nc.sync.dma_start(st["qkvT"][:, sl, :, :, :],
                                  st["qkv"][:, sl, :, :], transpose=True)

        def _rhs(st, t, i):
            # [din%128, (stile), s%128] strided view == [din, s] for chunk i
            return st["qkvT"][:, :, t, i, :]

        def emit_proj_qk_j(st, j):
            ps = mmps.tile([128, 512], F32, tag="mm", name=f"mmq_{j}")
            for i in range(NT):
                nc.tensor.matmul(
                    ps[:], wq_sb[:, i, 128 * j:128 * j + 128],
                    _rhs(st, 0, i), start=(i == 0), stop=(i == NT - 1))
            nc.scalar.activation(st["qT"][:, j, :], ps[:],
                                 AF.Identity, bias=bq5_sb[:, j:j + 1])
            ps = mmps.tile([128, 512], F32, tag="mm", name=f"mmk_{j}")
            for i in range(NT):
                nc.tensor.matmul(
                    ps[:], wk_sb[:, i, 128 * j:128 * j + 128],
                    _rhs(st, 1, i), start=(i == 0), stop=(i == NT - 1))
            nc.vector.tensor_copy(st["kT"][:, j, :], ps[:])

        def emit_proj_qk(st):
            for j in range(NT):
                emit_proj_qk_j(st, j)

        def emit_proj_v(st):
            for m in range(NT):
                ps = mmps.tile([128, 512], F32, tag="mm", name=f"mmv_{m}")
                for i in range(NT):
                    nc.tensor.matmul(
                        ps[:], st["qkvT"][:, m, 2, i, :],
                        wv_sb[:, i, :], start=(i == 0), stop=(i == NT - 1))
                if m % 2 == 0:
                    nc.scalar.activation(st["vN"][:, m, :], ps[:], AF.Copy)
                else:
                    nc.vector.tensor_copy(st["vN"][:, m, :], ps[:])

        # ---------------- attention ----------------
        def emit_scores(f, st, p2, si, a, norm_eng):
            s_ps = sps.tile([128, 2, 512], F32, tag="s",
                            name=f"s_{f}_{si}_{a}")
            for j, h in enumerate((2 * a, 2 * a + 1)):
                lo = 64 * (h % 2)
                nc.tensor.matmul(
                    s_ps[:, j, :],
                    st["qT"][lo:lo + 64, a, 128 * si:128 * si + 128],
                    st["kT"][lo:lo + 64, a, :], start=True, stop=True)
            m5 = statpool.tile([128, 2], F32, tag="m5",
                               name=f"m5_{f}_{si}_{a}")
            rs = statpool.tile([128, 2], F32, tag="rs",
                               name=f"rs_{f}_{si}_{a}")
            nc.vector.tensor_reduce(m5[:], s_ps[:], mybir.AxisListType.X,
                                    ALU.min)
            for j, h in enumerate((2 * a, 2 * a + 1)):
                nc.scalar.activation(p2[:, si % 2, h, :], s_ps[:, j, :],
                                     AF.Sigmoid, bias=m5[:, j:j + 1],
                                     scale=-1.0, accum_out=rs[:, j:j + 1])
            rsi = statpool.tile([128, 2], F32, tag="rsi",
                                name=f"rsi_{f}_{si}_{a}")
            nc.vector.reciprocal(rsi[:], rs[:])
            for j, h in enumerate((2 * a, 2 * a + 1)):
                norm_eng.tensor_scalar(p2[:, si % 2, h, :],
                                       p2[:, si % 2, h, :],
                                       rsi[:, j:j + 1], None, ALU.mult)

        def emit_attend_half(f, st, attnT_h, aT, half):
            """All 4 head pairs for one q-half (256 cols); two pairs share
            one psum bank."""
            for a in range(4):
                a_ps = attps.tile([128, 256], F32, tag="att",
                                  name=f"aps_{f}_{half}_{a}")
                for h in (2 * a, 2 * a + 1):
                    lo = 64 * (h % 2)
                    for kb in range(NT):
                        nc.tensor.matmul(
                            a_ps[lo:lo + 64, :],
                            st["vN"][:, kb, 64 * h:64 * h + 64],
                            attnT_h[:, :, 4 * h + kb, :],
                            start=(kb == 0), stop=(kb == NT - 1),
                            tile_position=(0, lo))
                nc.vector.tensor_copy(
                    aT[:, a, 256 * half:256 * half + 256], a_ps[:])

        def emit_outproj(f, aT, outsb, stp):
            ps = mmps.tile([128, 512], F32, tag="mm", name=f"mmo_{f}_{stp}")
            for j in range(NT):
                nc.tensor.matmul(
                    ps[:], aT[:, j, 128 * stp:128 * stp + 128],
                    wo_sb[:, j, :], start=(j == 0), stop=(j == NT - 1))
            if stp % 2 == 0:
                nc.scalar.activation(outsb[:, stp, :], ps[:], AF.Copy)
            else:
                nc.vector.tensor_copy(outsb[:, stp, :], ps[:])
            if stp == NT - 1:
                nc.gpsimd.dma_start(
                    out[f].rearrange("(a p) d -> p a d", p=128), outsb[:])

        # ---------------- schedule ----------------
        st0 = alloc_state(0)
        emit_load(0, st0)
        load_weights()
        # prime the sigmoid table set while DMAs run
        warm = wpool.tile([1, 2], F16)
        nc.vector.memset(warm[:], 0.0)
        nc.scalar.activation(warm[:], warm[:], AF.Sigmoid)
        emit_transpose_in(st0)
        # Q/K projection of frame 0 is interleaved per-j with si0's score
        # pairs inside the main loop (pair a only needs block j=a).

        st = st0
        prev = None  # (f, st, attnT_B, aT, outsb) awaiting half-B tail
        for f in range(FRAMES):
            nxt = alloc_state(f + 1) if f + 1 < FRAMES else None
            p2 = [ppool.tile([128, 2, H, 512], F16, tag=f"p{g}",
                             name=f"p_{f}_{g}") for g in range(2)]
            attnTs = [ppool.tile([128, 2, 32, 128], F16, tag=f"attnT{g}",
                                 name=f"attnT_{f}_{g}") for g in range(2)]
            aT = apool.tile([128, NT, 512], F16, tag="aT", name=f"aT_{f}")
            outsb = apool.tile([128, NT, 512], F16, tag="outsb",
                               name=f"outsb_{f}")
            fillers = {0: [], 1: [], 2: [], 3: []}
            if prev is not None:
                fp, stp, attnTBp, aTp, outsbp = prev
                fillers[0].append(
                    lambda: (emit_attend_half(fp, stp, attnTBp, aTp, 1),
                             emit_outproj(fp, aTp, outsbp, 2)))
                fillers[1].append(
                    lambda: emit_outproj(fp, aTp, outsbp, 3))
            if f == 0:
                fillers[0].append(lambda: emit_proj_v(st0))
            if nxt:
                fillers[0].append(lambda: emit_load(f + 1, nxt))
                fillers[1].append(lambda: emit_transpose_in(nxt))
                fillers[2].append(lambda: emit_proj_qk(nxt))
                fillers[3].append(lambda: emit_proj_v(nxt))
            for si in range(NT):
                for a in range(4):
                    if f == 0 and si == 0:
                        emit_proj_qk_j(st, a)
                    on_dve = (si == 0 and a < 2)
                    norm_eng = nc.vector if on_dve else nc.gpsimd
                    emit_scores(f, st, p2[si // 2], si, a, norm_eng)
                if nxt is None:
                    # last frame: quarter transposes shorten the tail
                    nc.sync.dma_start(attnTs[si // 2][:, si % 2, :, :],
                                      p2[si // 2][:, si % 2, :, :],
                                      transpose=True)
                elif si % 2 == 1:
                    # transpose the completed half (si-1, si)
                    nc.sync.dma_start(attnTs[si // 2][:],
                                      p2[si // 2][:], transpose=True)
                for fn in fillers[si]:
                    fn()
            # half-A attend + its out-proj columns still inside frame f
            emit_attend_half(f, st, attnTs[0], aT, 0)
            emit_outproj(f, aT, outsb, 0)
            emit_outproj(f, aT, outsb, 1)
            prev = (f, st, attnTs[1], aT, outsb)
            st = nxt
        fp, stp, attnTBp, aTp, outsbp = prev
        emit_attend_half(fp, stp, attnTBp, aTp, 1)
        emit_outproj(fp, aTp, outsbp, 2)
        emit_outproj(fp, aTp, outsbp, 3)


def build_nc():
    nc = bacc.Bacc("TRN2", target_bir_lowering=False, debug=False,
                   num_devices=NCORES)
    qkv = nc.dram_tensor("qkv", (FRAMES, 128, NT, 3, D), F32,
                         kind="ExternalInput").ap()
    wall = nc.dram_tensor("wall", (128, 4, NT, D), F16,
                          kind="ExternalInput").ap()
    bq5 = nc.dram_tensor("bq5", (D,), F32, kind="ExternalInput").ap()
    out = nc.dram_tensor("out", (FRAMES, S, D), F16,
                         kind="ExternalOutput").ap()
    with tile.TileContext(nc) as tc:
        _emit(tc, nc, (qkv, wall, bq5, out))
    nc.compile()
    return nc


_NC = None


def _get_nc():
    global _NC
    if _NC is None:
        _NC = build_nc()
    return _NC


def make_in_maps(query_spikes, key_spikes, value_spikes, Wq, bq, Wk, bk,
                 Wv, bv, Wo, bo, modality_weights, temporal_sync,
                 query_modality, key_modality):
    qm = int(query_modality)
    km = int(key_modality)
    mw = np.asarray(modality_weights, np.float32)
    c = (mw[qm] * mw[km]) / np.float32(math.sqrt(HD))  # [H]
    scale_cols = np.repeat(-5.0 * c, HD).astype(np.float32)  # [D]
    wq_s = np.asarray(Wq, np.float32) * scale_cols[None, :]
    bq5 = (np.asarray(bq, np.float32) * scale_cols).astype(np.float32)

    f16 = lambda a: np.asarray(a, np.float32).astype(np.float16)
    # wall[p, w, i, n] = W_w[i*128+p, n]
    wall = np.stack([f16(wq_s), f16(Wk), f16(Wv), f16(Wo)])  # [4, D, D]
    wall = np.ascontiguousarray(
        wall.reshape(4, NT, 128, D).transpose(2, 0, 1, 3))
    shared = {"wall": wall, "bq5": bq5}
    # qkv_all[f, p, a, t, d] = tensor_t[f, a*128+p, d]
    qkv_all = np.stack([
        np.asarray(query_spikes, np.float32).reshape(B * T, S, D),
        np.asarray(key_spikes, np.float32).reshape(B * T, S, D),
        np.asarray(value_spikes, np.float32).reshape(B * T, S, D),
    ], axis=1)  # [B*T, 3, S, D]
    qkv_all = qkv_all.reshape(B * T, 3, NT, 128, D).transpose(0, 3, 2, 1, 4)
    in_maps = []
    for core in range(NCORES):
        sl = slice(core * FRAMES, (core + 1) * FRAMES)
        in_maps.append({
            "qkv": np.ascontiguousarray(qkv_all[sl]),
            **shared,
        })
    return in_maps


def host_bias(Wv_np, bv_np, Wo_np, bo_np):
    return (np.asarray(bv_np, np.float64) @ np.asarray(Wo_np, np.float64)
            + np.asarray(bo_np, np.float64)).astype(np.float32)


def kernel(**inputs):
    nc = _get_nc()
    in_maps = make_in_maps(**inputs)
    res = bass_utils.run_bass_kernel_spmd(
        nc, in_maps, core_ids=list(range(NCORES)))
    out = np.concatenate([np.asarray(r["out"], np.float16).astype(np.float32)
                          for r in res.results], axis=0)
    out += host_bias(inputs["Wv"], inputs["bv"], inputs["Wo"], inputs["bo"])
    return out.reshape(B, T, S, D)


# revision 5
# speedup vs baseline: 1.2829x; 1.0017x over previous
"""CrossModalAttention Trainium2 kernel, v3.

Data-parallel over B*T = 32 frames -> 4 frames per core on 8 cores.
fp16 on-chip; f32 PSUM/stats.  Host-side algebra (see v2): bk drops, bq
folds into the Q projection evac bias, -5*mw/sqrt(hd) folds into Wq, the
sigmoid bias is the DVE rowmin, bv/bo become a constant row added in
numpy.  The DMA-transpose/DMA serialization in the scheduler makes DMA
op COUNT the scarce resource, so v3 uses one packed qkv load, ONE
whole-frame input transpose, TWO half-frame attn transposes and one
store per frame (5 DMA ops total).
"""

import math

import numpy as np

import concourse.bass as bass
import concourse.bacc as bacc
import concourse.mybir as mybir
import concourse.tile as tile
from concourse import bass_utils

F16 = mybir.dt.float16
F32 = mybir.dt.float32
AF = mybir.ActivationFunctionType
ALU = mybir.AluOpType

B, T, S, D = 2, 16, 512, 512
H, HD = 8, 64
NCORES = 8
FRAMES = B * T // NCORES  # 4 frames per core
NT = D // 128


def _emit(tc, nc, aps):
    qkv, wall, bq5, out = aps

    with tc.tile_pool(name="wpool", bufs=1) as wpool, \
         tc.tile_pool(name="inpool", bufs=2) as inpool, \
         tc.tile_pool(name="tpool", bufs=2) as tpool, \
         tc.tile_pool(name="projpool", bufs=2) as projpool, \
         tc.tile_pool(name="ppool", bufs=1) as ppool, \
         tc.tile_pool(name="apool", bufs=2) as apool, \
         tc.tile_pool(name="statpool", bufs=16) as statpool, \
         tc.tile_pool(name="sps", bufs=2, space="PSUM") as sps, \
         tc.tile_pool(name="attps", bufs=2, space="PSUM") as attps, \
         tc.tile_pool(name="mmps", bufs=2, space="PSUM") as mmps:

        # ---------------- weights (one-time, one packed DMA) -------------
        walls = wpool.tile([128, 4, NT, 512], F16)  # w, i, n
        bq5_sb = wpool.tile([128, NT], F32)
        wq_sb = walls[:, 0, :, :]
        wk_sb = walls[:, 1, :, :]
        wv_sb = walls[:, 2, :, :]
        wo_sb = walls[:, 3, :, :]

        def load_weights():
            nc.gpsimd.dma_start(walls[:], wall[:])
            nc.gpsimd.dma_start(bq5_sb[:],
                                bq5.rearrange("(i p) -> p i", p=128))

        # ---------------- per-frame state ----------------
        def alloc_state(f):
            st = {}
            # natural: [s%128, stile, tensor, d] -- one contiguous region
            st["qkv"] = inpool.tile([128, NT, 3, 512], F16, tag="qkv",
                                    name=f"qkv_{f}")
            # transposed: [d%128, stile, tensor, dblk, s%128]
            st["qkvT"] = tpool.tile([128, NT, 3, NT, 128], F16, tag="qkvT",
                                    name=f"qkvT_{f}")
            # projections: [dout%128 (head pair-stacked), pair, s]
            st["qT"] = projpool.tile([128, NT, 512], F16, tag="qT",
                                     name=f"qT_{f}")
            st["kT"] = projpool.tile([128, NT, 512], F16, tag="kT",
                                     name=f"kT_{f}")
            # V natural: [k%128, kblk, dout]
            st["vN"] = projpool.tile([128, NT, 512], F16, tag="vN", bufs=3,
                                     name=f"vN_{f}")
            return st

        def emit_load(f, st, half=None):
            if half is None:
                nc.gpsimd.dma_start(st["qkv"][:], qkv[f])
            else:
                sl = slice(2 * half, 2 * half + 2)
                nc.gpsimd.dma_start(st["qkv"][:, sl, :, :], qkv[f][:, sl, :, :])

        def emit_transpose_in(st, half=None):
            if half is None:
                nc.sync.dma_start(st["qkvT"][:], st["qkv"][:], transpose=True)
            else:
                sl = slice(2 * half, 2 * half + 2)
                nc.sync.dma_start(st["qkvT"][:, sl, :, :, :],
                                  st["qkv"][:, sl, :, :], transpose=True)

        def _rhs(st, t, i):
            # [din%128, (stile), s%128] strided view == [din, s] for chunk i
            return st["qkvT"][:, :, t, i, :]

        def emit_proj_qk_j(st, j):
            ps = mmps.tile([128, 512], F32, tag="mm", name=f"mmq_{j}")
            for i in range(NT):
                nc.tensor.matmul(
                    ps[:], wq_sb[:, i, 128 * j:128 * j + 128],
                    _rhs(st, 0, i), start=(i == 0), stop=(i == NT - 1))
            nc.scalar.activation(st["qT"][:, j, :], ps[:],
                                 AF.Identity, bias=bq5_sb[:, j:j + 1])
            ps = mmps.tile([128, 512], F32, tag="mm", name=f"mmk_{j}")
            for i in range(NT):
                nc.tensor.matmul(
                    ps[:], wk_sb[:, i, 128 * j:128 * j + 128],
                    _rhs(st, 1, i), start=(i == 0), stop=(i == NT - 1))
            nc.vector.tensor_copy(st["kT"][:, j, :], ps[:])

        def emit_proj_qk(st):
            for j in range(NT):
                emit_proj_qk_j(st, j)

        def emit_proj_v(st):
            for m in range(NT):
                ps = mmps.tile([128, 512], F32, tag="mm", name=f"mmv_{m}")
                for i in range(NT):
                    nc.tensor.matmul(
                        ps[:], st["qkvT"][:, m, 2, i, :],
                        wv_sb[:, i, :], start=(i == 0), stop=(i == NT - 1))
                if m % 2 == 0:
                    nc.scalar.activation(st["vN"][:, m, :], ps[:], AF.Copy)
                else:
                    nc.vector.tensor_copy(st["vN"][:, m, :], ps[:])

        # ---------------- attention ----------------
        def emit_scores(f, st, p2, si, a, norm_eng):
            s_ps = sps.tile([128, 2, 512], F32, tag="s",
                            name=f"s_{f}_{si}_{a}")
            for j, h in enumerate((2 * a, 2 * a + 1)):
                lo = 64 * (h % 2)
                nc.tensor.matmul(
                    s_ps[:, j, :],
                    st["qT"][lo:lo + 64, a, 128 * si:128 * si + 128],
                    st["kT"][lo:lo + 64, a, :], start=True, stop=True)
            m5 = statpool.tile([128, 2], F32, tag="m5",
                               name=f"m5_{f}_{si}_{a}")
            rs = statpool.tile([128, 2], F32, tag="rs",
                               name=f"rs_{f}_{si}_{a}")
            nc.vector.tensor_reduce(m5[:], s_ps[:], mybir.AxisListType.X,
                                    ALU.min)
            for j, h in enumerate((2 * a, 2 * a + 1)):
                nc.scalar.activation(p2[:, si % 2, h, :], s_ps[:, j, :],
                                     AF.Sigmoid, bias=m5[:, j:j + 1],
                                     scale=-1.0, accum_out=rs[:, j:j + 1])
            rsi = statpool.tile([128, 2], F32, tag="rsi",
                                name=f"rsi_{f}_{si}_{a}")
            nc.vector.reciprocal(rsi[:], rs[:])
            for j, h in enumerate((2 * a, 2 * a + 1)):
                norm_eng.tensor_scalar(p2[:, si % 2, h, :],
                                       p2[:, si % 2, h, :],
                                       rsi[:, j:j + 1], None, ALU.mult)

        def emit_attend_half(f, st, attnT_h, aT, half):
            """All 4 head pairs for one q-half (256 cols); two pairs share
            one psum bank."""
            for a in range(4):
                a_ps = attps.tile([128, 256], F32, tag="att",
                                  name=f"aps_{f}_{half}_{a}")
                for h in (2 * a, 2 * a + 1):
                    lo = 64 * (h % 2)
                    for kb in range(NT):
                        nc.tensor.matmul(
                            a_ps[lo:lo + 64, :],
                            st["vN"][:, kb, 64 * h:64 * h + 64],
                            attnT_h[:, :, 4 * h + kb, :],
                            start=(kb == 0), stop=(kb == NT - 1),
                            tile_position=(0, lo))
                nc.vector.tensor_copy(
                    aT[:, a, 256 * half:256 * half + 256], a_ps[:])

        def emit_outproj(f, aT, outsb, stp):
            ps = mmps.tile([128, 512], F32, tag="mm", name=f"mmo_{f}_{stp}")
            for j in range(NT):
                nc.tensor.matmul(
                    ps[:], aT[:, j, 128 * stp:128 * stp + 128],
                    wo_sb[:, j, :], start=(j == 0), stop=(j == NT - 1))
            if stp % 2 == 0:
                nc.scalar.activation(outsb[:, stp, :], ps[:], AF.Copy)
            else:
                nc.vector.tensor_copy(outsb[:, stp, :], ps[:])
            if stp == NT - 1:
                nc.gpsimd.dma_start(
                    out[f].rearrange("(a p) d -> p a d", p=128), outsb[:])

        # ---------------- schedule ----------------
        st0 = alloc_state(0)
        emit_load(0, st0)
        load_weights()
        # prime the sigmoid table set while DMAs run
        warm = wpool.tile([1, 2], F16)
        nc.vector.memset(warm[:], 0.0)
        nc.scalar.activation(warm[:], warm[:], AF.Sigmoid)
        emit_transpose_in(st0)
        # Q/K projection of frame 0 is interleaved per-j with si0's score
        # pairs inside the main loop (pair a only needs block j=a).

        st = st0
        prev = None  # (f, st, attnT_B, aT, outsb) awaiting half-B tail
        for f in range(FRAMES):
            nxt = alloc_state(f + 1) if f + 1 < FRAMES else None
            p2 = [ppool.tile([128, 2, H, 512], F16, tag=f"p{g}",
                             name=f"p_{f}_{g}") for g in range(2)]
            attnTs = [ppool.tile([128, 2, 32, 128], F16, tag=f"attnT{g}",
                                 name=f"attnT_{f}_{g}") for g in range(2)]
            aT = apool.tile([128, NT, 512], F16, tag="aT", name=f"aT_{f}")
            outsb = apool.tile([128, NT, 512], F16, tag="outsb",
                               name=f"outsb_{f}")
            fillers = {0: [], 1: [], 2: [], 3: []}
            if prev is not None:
                fp, stp, attnTBp, aTp, outsbp = prev
                fillers[0].append(
                    lambda: (emit_attend_half(fp, stp, attnTBp, aTp, 1),
                             emit_outproj(fp, aTp, outsbp, 2)))
                fillers[1].append(
                    lambda: emit_outproj(fp, aTp, outsbp, 3))
            if f == 0:
                fillers[0].append(lambda: emit_proj_v(st0))
            if nxt:
                fillers[0].append(lambda: emit_load(f + 1, nxt))
                fillers[1].append(lambda: emit_transpose_in(nxt))
                fillers[2].append(lambda: emit_proj_qk(nxt))
                fillers[3].append(lambda: emit_proj_v(nxt))
            for si in range(NT):
                for a in range(4):
                    if f == 0 and si == 0:
                        emit_proj_qk_j(st, a)
                    on_dve = (si % 2 == 0 and a < 2)
                    norm_eng = nc.vector if on_dve else nc.gpsimd
                    emit_scores(f, st, p2[si // 2], si, a, norm_eng)
                if nxt is None:
                    # last frame: quarter transposes shorten the tail
                    nc.sync.dma_start(attnTs[si // 2][:, si % 2, :, :],
                                      p2[si // 2][:, si % 2, :, :],
                                      transpose=True)
                elif si % 2 == 1:
                    # transpose the completed half (si-1, si)
                    nc.sync.dma_start(attnTs[si // 2][:],
                                      p2[si // 2][:], transpose=True)
                for fn in fillers[si]:
                    fn()
            # half-A attend + its out-proj columns still inside frame f
            emit_attend_half(f, st, attnTs[0], aT, 0)
            emit_outproj(f, aT, outsb, 0)
            emit_outproj(f, aT, outsb, 1)
            prev = (f, st, attnTs[1], aT, outsb)
            st = nxt
        fp, stp, attnTBp, aTp, outsbp = prev
        emit_attend_half(fp, stp, attnTBp, aTp, 1)
        emit_outproj(fp, aTp, outsbp, 2)
        emit_outproj(fp, aTp, outsbp, 3)


def build_nc():
    nc = bacc.Bacc("TRN2", target_bir_lowering=False, debug=False,
                   num_devices=NCORES)
    qkv = nc.dram_tensor("qkv", (FRAMES, 128, NT, 3, D), F32,
                         kind="ExternalInput").ap()
    wall = nc.dram_tensor("wall", (128, 4, NT, D), F16,
                          kind="ExternalInput").ap()
    bq5 = nc.dram_tensor("bq5", (D,), F32, kind="ExternalInput").ap()
    out = nc.dram_tensor("out", (FRAMES, S, D), F16,
                         kind="ExternalOutput").ap()
    with tile.TileContext(nc) as tc:
        _emit(tc, nc, (qkv, wall, bq5, out))
    nc.compile()
    return nc


_NC = None


def _get_nc():
    global _NC
    if _NC is None:
        _NC = build_nc()
    return _NC


def make_in_maps(query_spikes, key_spikes, value_spikes, Wq, bq, Wk, bk,
                 Wv, bv, Wo, bo, modality_weights, temporal_sync,
                 query_modality, key_modality):
    qm = int(query_modality)
    km = int(key_modality)
    mw = np.asarray(modality_weights, np.float32)
    c = (mw[qm] * mw[km]) / np.float32(math.sqrt(HD))  # [H]
    scale_cols = np.repeat(-5.0 * c, HD).astype(np.float32)  # [D]
    wq_s = np.asarray(Wq, np.float32) * scale_cols[None, :]
    bq5 = (np.asarray(bq, np.float32) * scale_cols).astype(np.float32)

    f16 = lambda a: np.asarray(a, np.float32).astype(np.float16)
    # wall[p, w, i, n] = W_w[i*128+p, n]
    wall = np.stack([f16(wq_s), f16(Wk), f16(Wv), f16(Wo)])  # [4, D, D]
    wall = np.ascontiguousarray(
        wall.reshape(4, NT, 128, D).transpose(2, 0, 1, 3))
    shared = {"wall": wall, "bq5": bq5}
    # qkv_all[f, p, a, t, d] = tensor_t[f, a*128+p, d]
    qkv_all = np.stack([
        np.asarray(query_spikes, np.float32).reshape(B * T, S, D),
        np.asarray(key_spikes, np.float32).reshape(B * T, S, D),
        np.asarray(value_spikes, np.float32).reshape(B * T, S, D),
    ], axis=1)  # [B*T, 3, S, D]
    qkv_all = qkv_all.reshape(B * T, 3, NT, 128, D).transpose(0, 3, 2, 1, 4)
    in_maps = []
    for core in range(NCORES):
        sl = slice(core * FRAMES, (core + 1) * FRAMES)
        in_maps.append({
            "qkv": np.ascontiguousarray(qkv_all[sl]),
            **shared,
        })
    return in_maps


def host_bias(Wv_np, bv_np, Wo_np, bo_np):
    return (np.asarray(bv_np, np.float64) @ np.asarray(Wo_np, np.float64)
            + np.asarray(bo_np, np.float64)).astype(np.float32)


def kernel(**inputs):
    nc = _get_nc()
    in_maps = make_in_maps(**inputs)
    res = bass_utils.run_bass_kernel_spmd(
        nc, in_maps, core_ids=list(range(NCORES)))
    out = np.concatenate([np.asarray(r["out"], np.float16).astype(np.float32)
                          for r in res.results], axis=0)
    out += host_bias(inputs["Wv"], inputs["bv"], inputs["Wo"], inputs["bo"])
    return out.reshape(B, T, S, D)


# revision 7
# speedup vs baseline: 1.3485x; 1.0512x over previous
"""CrossModalAttention Trainium2 kernel, v3.

Data-parallel over B*T = 32 frames -> 4 frames per core on 8 cores.
fp16 on-chip; f32 PSUM/stats.  Host-side algebra (see v2): bk drops, bq
folds into the Q projection evac bias, -5*mw/sqrt(hd) folds into Wq, the
sigmoid bias is the DVE rowmin, bv/bo become a constant row added in
numpy.  The DMA-transpose/DMA serialization in the scheduler makes DMA
op COUNT the scarce resource, so v3 uses one packed qkv load, ONE
whole-frame input transpose, TWO half-frame attn transposes and one
store per frame (5 DMA ops total).
"""

import math

import numpy as np

import concourse.bass as bass
import concourse.bacc as bacc
import concourse.mybir as mybir
import concourse.tile as tile
from concourse import bass_utils

F16 = mybir.dt.float16
F32 = mybir.dt.float32
AF = mybir.ActivationFunctionType
ALU = mybir.AluOpType

B, T, S, D = 2, 16, 512, 512
H, HD = 8, 64
NCORES = 8
FRAMES = B * T // NCORES  # 4 frames per core
NT = D // 128


def _emit(tc, nc, aps):
    qkv, wall, bq5, out = aps

    with tc.tile_pool(name="wpool", bufs=1) as wpool, \
         tc.tile_pool(name="tpool", bufs=2) as tpool, \
         tc.tile_pool(name="projpool", bufs=2) as projpool, \
         tc.tile_pool(name="ppool", bufs=1) as ppool, \
         tc.tile_pool(name="apool", bufs=2) as apool, \
         tc.tile_pool(name="statpool", bufs=16) as statpool, \
         tc.tile_pool(name="sps", bufs=2, space="PSUM") as sps, \
         tc.tile_pool(name="attps", bufs=2, space="PSUM") as attps, \
         tc.tile_pool(name="mmps", bufs=2, space="PSUM") as mmps:

        # ---------------- weights (one-time, one packed DMA) -------------
        walls = wpool.tile([128, 4, NT, 512], F16)  # w, i, n
        bq5_sb = wpool.tile([128, NT], F32)
        wq_sb = walls[:, 0, :, :]
        wk_sb = walls[:, 1, :, :]
        wv_sb = walls[:, 2, :, :]
        wo_sb = walls[:, 3, :, :]

        def load_weights(group):
            sl = slice(2 * group, 2 * group + 2)
            nc.gpsimd.dma_start(walls[:, sl, :, :], wall[:, sl, :, :])
            if group == 0:
                nc.gpsimd.dma_start(bq5_sb[:],
                                    bq5.rearrange("(i p) -> p i", p=128))

        # ---------------- per-frame state ----------------
        def alloc_state(f):
            st = {}
            # transposed: [d%128, stile, tensor, dblk, s%128] -- loaded
            # directly from the host-pre-transposed fp16 DRAM layout
            st["qkvT"] = tpool.tile([128, NT, 3, NT, 128], F16, tag="qkvT",
                                    name=f"qkvT_{f}")
            # projections: [dout%128 (head pair-stacked), pair, s]
            st["qT"] = projpool.tile([128, NT, 512], F16, tag="qT",
                                     name=f"qT_{f}")
            st["kT"] = projpool.tile([128, NT, 512], F16, tag="kT",
                                     name=f"kT_{f}")
            # V natural: [k%128, kblk, dout]
            st["vN"] = projpool.tile([128, NT, 512], F16, tag="vN", bufs=3,
                                     name=f"vN_{f}")
            return st

        def emit_load(f, st):
            nc.gpsimd.dma_start(st["qkvT"][:], qkv[f])

        def _rhs(st, t, i):
            # [din%128, (stile), s%128] strided view == [din, s] for chunk i
            return st["qkvT"][:, :, t, i, :]

        def emit_proj_qk_j(st, j):
            ps = mmps.tile([128, 512], F32, tag="mm", name=f"mmq_{j}")
            for i in range(NT):
                nc.tensor.matmul(
                    ps[:], wq_sb[:, i, 128 * j:128 * j + 128],
                    _rhs(st, 0, i), start=(i == 0), stop=(i == NT - 1))
            nc.scalar.activation(st["qT"][:, j, :], ps[:],
                                 AF.Identity, bias=bq5_sb[:, j:j + 1])
            ps = mmps.tile([128, 512], F32, tag="mm", name=f"mmk_{j}")
            for i in range(NT):
                nc.tensor.matmul(
                    ps[:], wk_sb[:, i, 128 * j:128 * j + 128],
                    _rhs(st, 1, i), start=(i == 0), stop=(i == NT - 1))
            nc.vector.tensor_copy(st["kT"][:, j, :], ps[:])

        def emit_proj_qk(st):
            for j in range(NT):
                emit_proj_qk_j(st, j)

        def emit_proj_v(st):
            for m in range(NT):
                ps = mmps.tile([128, 512], F32, tag="mm", name=f"mmv_{m}")
                for i in range(NT):
                    nc.tensor.matmul(
                        ps[:], st["qkvT"][:, m, 2, i, :],
                        wv_sb[:, i, :], start=(i == 0), stop=(i == NT - 1))
                if m % 2 == 0:
                    nc.scalar.activation(st["vN"][:, m, :], ps[:], AF.Copy)
                else:
                    nc.vector.tensor_copy(st["vN"][:, m, :], ps[:])

        # ---------------- attention ----------------
        def emit_scores(f, st, p2, si, a, norm_eng):
            s_ps = sps.tile([128, 2, 512], F32, tag="s",
                            name=f"s_{f}_{si}_{a}")
            for j, h in enumerate((2 * a, 2 * a + 1)):
                lo = 64 * (h % 2)
                nc.tensor.matmul(
                    s_ps[:, j, :],
                    st["qT"][lo:lo + 64, a, 128 * si:128 * si + 128],
                    st["kT"][lo:lo + 64, a, :], start=True, stop=True)
            m5 = statpool.tile([128, 2], F32, tag="m5",
                               name=f"m5_{f}_{si}_{a}")
            rs = statpool.tile([128, 2], F32, tag="rs",
                               name=f"rs_{f}_{si}_{a}")
            nc.vector.tensor_reduce(m5[:], s_ps[:], mybir.AxisListType.X,
                                    ALU.min)
            for j, h in enumerate((2 * a, 2 * a + 1)):
                nc.scalar.activation(p2[:, si % 2, h, :], s_ps[:, j, :],
                                     AF.Sigmoid, bias=m5[:, j:j + 1],
                                     scale=-1.0, accum_out=rs[:, j:j + 1])
            rsi = statpool.tile([128, 2], F32, tag="rsi",
                                name=f"rsi_{f}_{si}_{a}")
            nc.vector.reciprocal(rsi[:], rs[:])
            for j, h in enumerate((2 * a, 2 * a + 1)):
                norm_eng.tensor_scalar(p2[:, si % 2, h, :],
                                       p2[:, si % 2, h, :],
                                       rsi[:, j:j + 1], None, ALU.mult)

        def emit_attend_half(f, st, attnT_h, aT, half):
            """All 4 head pairs for one q-half (256 cols); two pairs share
            one psum bank."""
            for a in range(4):
                a_ps = attps.tile([128, 256], F32, tag="att",
                                  name=f"aps_{f}_{half}_{a}")
                for h in (2 * a, 2 * a + 1):
                    lo = 64 * (h % 2)
                    for kb in range(NT):
                        nc.tensor.matmul(
                            a_ps[lo:lo + 64, :],
                            st["vN"][:, kb, 64 * h:64 * h + 64],
                            attnT_h[:, :, 4 * h + kb, :],
                            start=(kb == 0), stop=(kb == NT - 1),
                            tile_position=(0, lo))
                nc.vector.tensor_copy(
                    aT[:, a, 256 * half:256 * half + 256], a_ps[:])

        def emit_attend_quarter(f, st, attnT_h, aT, half, q2):
            for a in range(4):
                a_ps = attps.tile([128, 256], F32, tag="att",
                                  name=f"apsq_{f}_{half}_{q2}_{a}")
                for h in (2 * a, 2 * a + 1):
                    lo = 64 * (h % 2)
                    for kb in range(NT):
                        nc.tensor.matmul(
                            a_ps[lo:lo + 64, 0:128],
                            st["vN"][:, kb, 64 * h:64 * h + 64],
                            attnT_h[:, q2, 4 * h + kb, :],
                            start=(kb == 0), stop=(kb == NT - 1),
                            tile_position=(0, lo))
                nc.vector.tensor_copy(
                    aT[:, a, 256 * half + 128 * q2:
                       256 * half + 128 * q2 + 128], a_ps[:, 0:128])

        def emit_outproj(f, aT, outsb, stp):
            ps = mmps.tile([128, 512], F32, tag="mm", name=f"mmo_{f}_{stp}")
            for j in range(NT):
                nc.tensor.matmul(
                    ps[:], aT[:, j, 128 * stp:128 * stp + 128],
                    wo_sb[:, j, :], start=(j == 0), stop=(j == NT - 1))
            if stp % 2 == 0:
                nc.scalar.activation(outsb[:, stp, :], ps[:], AF.Copy)
            else:
                nc.vector.tensor_copy(outsb[:, stp, :], ps[:])
            if stp == NT - 1:
                nc.gpsimd.dma_start(
                    out[f].rearrange("(a p) d -> p a d", p=128), outsb[:])

        # ---------------- schedule ----------------
        st0 = alloc_state(0)
        load_weights(0)
        emit_load(0, st0)
        load_weights(1)
        # prime the sigmoid table set while DMAs run
        warm = wpool.tile([1, 2], F16)
        nc.vector.memset(warm[:], 0.0)
        nc.scalar.activation(warm[:], warm[:], AF.Sigmoid)
        # Q/K projection of frame 0 is interleaved per-j with si0's score
        # pairs inside the main loop (pair a only needs block j=a).

        st = st0
        prev = None  # (f, st, attnT_B, aT, outsb) awaiting half-B tail
        for f in range(FRAMES):
            nxt = alloc_state(f + 1) if f + 1 < FRAMES else None
            p2 = [ppool.tile([128, 2, H, 512], F16, tag=f"p{g}",
                             name=f"p_{f}_{g}") for g in range(2)]
            attnTs = [ppool.tile([128, 2, 32, 128], F16, tag=f"attnT{g}",
                                 name=f"attnT_{f}_{g}") for g in range(2)]
            aT = apool.tile([128, NT, 512], F16, tag="aT", name=f"aT_{f}")
            outsb = apool.tile([128, NT, 512], F16, tag="outsb",
                               name=f"outsb_{f}")
            fillers = {0: [], 1: [], 2: [], 3: []}
            if prev is not None:
                fp, stp, attnTsp, aTp, outsbp = prev
                fillers[0].append(
                    lambda: (emit_attend_half(fp, stp, attnTsp[0], aTp, 0),
                             emit_outproj(fp, aTp, outsbp, 0),
                             emit_outproj(fp, aTp, outsbp, 1)))
                fillers[1].append(
                    lambda: (emit_attend_half(fp, stp, attnTsp[1], aTp, 1),
                             emit_outproj(fp, aTp, outsbp, 2),
                             emit_outproj(fp, aTp, outsbp, 3)))
            if f == 0:
                fillers[0].append(lambda: emit_proj_v(st0))
            if nxt:
                fillers[1].append(lambda: emit_load(f + 1, nxt))
                fillers[2].append(lambda: emit_proj_qk(nxt))
                fillers[3].append(lambda: emit_proj_v(nxt))
            for si in range(NT):
                for a in range(4):
                    if f == 0 and si == 0:
                        emit_proj_qk_j(st, a)
                    on_dve = (si % 2 == 0 and a < 2)
                    norm_eng = nc.vector if on_dve else nc.gpsimd
                    emit_scores(f, st, p2[si // 2], si, a, norm_eng)
                if nxt is None:
                    # last frame: quarter transposes shorten the tail
                    nc.sync.dma_start(attnTs[si // 2][:, si % 2, :, :],
                                      p2[si // 2][:, si % 2, :, :],
                                      transpose=True)
                elif si % 2 == 1:
                    # transpose the completed half (si-1, si)
                    nc.sync.dma_start(attnTs[si // 2][:],
                                      p2[si // 2][:], transpose=True)
                for fn in fillers[si]:
                    fn()
                if nxt is None and si == 2:
                    emit_attend_half(f, st, attnTs[0], aT, 0)
                    emit_outproj(f, aT, outsb, 0)
                    emit_outproj(f, aT, outsb, 1)
                if nxt is None and si == 3:
                    emit_attend_quarter(f, st, attnTs[1], aT, 1, 0)
                    emit_outproj(f, aT, outsb, 2)
            if nxt is None:
                emit_attend_quarter(f, st, attnTs[1], aT, 1, 1)
                emit_outproj(f, aT, outsb, 3)
            prev = (f, st, attnTs, aT, outsb)
            st = nxt


def build_nc():
    nc = bacc.Bacc("TRN2", target_bir_lowering=False, debug=False,
                   num_devices=NCORES)
    qkv = nc.dram_tensor("qkv", (FRAMES, 128, NT, 3, NT, 128), F16,
                         kind="ExternalInput").ap()
    wall = nc.dram_tensor("wall", (128, 4, NT, D), F16,
                          kind="ExternalInput").ap()
    bq5 = nc.dram_tensor("bq5", (D,), F32, kind="ExternalInput").ap()
    out = nc.dram_tensor("out", (FRAMES, S, D), F16,
                         kind="ExternalOutput").ap()
    with tile.TileContext(nc) as tc:
        _emit(tc, nc, (qkv, wall, bq5, out))
    nc.compile()
    return nc


_NC = None


def _get_nc():
    global _NC
    if _NC is None:
        _NC = build_nc()
    return _NC


def make_in_maps(query_spikes, key_spikes, value_spikes, Wq, bq, Wk, bk,
                 Wv, bv, Wo, bo, modality_weights, temporal_sync,
                 query_modality, key_modality):
    qm = int(query_modality)
    km = int(key_modality)
    mw = np.asarray(modality_weights, np.float32)
    c = (mw[qm] * mw[km]) / np.float32(math.sqrt(HD))  # [H]
    scale_cols = np.repeat(-5.0 * c, HD).astype(np.float32)  # [D]
    wq_s = np.asarray(Wq, np.float32) * scale_cols[None, :]
    bq5 = (np.asarray(bq, np.float32) * scale_cols).astype(np.float32)

    f16 = lambda a: np.asarray(a, np.float32).astype(np.float16)
    # wall[p, w, i, n] = W_w[i*128+p, n]
    wall = np.stack([f16(wq_s), f16(Wk), f16(Wv), f16(Wo)])  # [4, D, D]
    wall = np.ascontiguousarray(
        wall.reshape(4, NT, 128, D).transpose(2, 0, 1, 3))
    shared = {"wall": wall, "bq5": bq5}
    # qkv_all[f, dp, st, t, db, sl] = tensor_t[f, st*128+sl, db*128+dp]
    qkv_all = np.stack([
        np.asarray(query_spikes, np.float32).reshape(B * T, S, D),
        np.asarray(key_spikes, np.float32).reshape(B * T, S, D),
        np.asarray(value_spikes, np.float32).reshape(B * T, S, D),
    ], axis=1).astype(np.float16)  # [B*T, 3, S, D]
    qkv_all = qkv_all.reshape(B * T, 3, NT, 128, NT, 128).transpose(
        0, 5, 2, 1, 4, 3)
    in_maps = []
    for core in range(NCORES):
        sl = slice(core * FRAMES, (core + 1) * FRAMES)
        in_maps.append({
            "qkv": np.ascontiguousarray(qkv_all[sl]),
            **shared,
        })
    return in_maps


def host_bias(Wv_np, bv_np, Wo_np, bo_np):
    return (np.asarray(bv_np, np.float64) @ np.asarray(Wo_np, np.float64)
            + np.asarray(bo_np, np.float64)).astype(np.float32)


def kernel(**inputs):
    nc = _get_nc()
    in_maps = make_in_maps(**inputs)
    res = bass_utils.run_bass_kernel_spmd(
        nc, in_maps, core_ids=list(range(NCORES)))
    out = np.concatenate([np.asarray(r["out"], np.float16).astype(np.float32)
                          for r in res.results], axis=0)
    out += host_bias(inputs["Wv"], inputs["bv"], inputs["Wo"], inputs["bo"])
    return out.reshape(B, T, S, D)


# revision 9
# speedup vs baseline: 1.4263x; 1.0577x over previous
"""CrossModalAttention Trainium2 kernel, v3.

Data-parallel over B*T = 32 frames -> 4 frames per core on 8 cores.
fp16 on-chip; f32 PSUM/stats.  Host-side algebra (see v2): bk drops, bq
folds into the Q projection evac bias, -5*mw/sqrt(hd) folds into Wq, the
sigmoid bias is the DVE rowmin, bv/bo become a constant row added in
numpy.  The DMA-transpose/DMA serialization in the scheduler makes DMA
op COUNT the scarce resource, so v3 uses one packed qkv load, ONE
whole-frame input transpose, TWO half-frame attn transposes and one
store per frame (5 DMA ops total).
"""

import math

import numpy as np

import concourse.bass as bass
import concourse.bacc as bacc
import concourse.mybir as mybir
import concourse.tile as tile
from concourse import bass_utils

F16 = mybir.dt.float16
F32 = mybir.dt.float32
AF = mybir.ActivationFunctionType
ALU = mybir.AluOpType

B, T, S, D = 2, 16, 512, 512
H, HD = 8, 64
NCORES = 8
FRAMES = B * T // NCORES  # 4 frames per core
NT = D // 128


def _emit(tc, nc, aps):
    qkv, wall, bq5, out = aps

    with tc.tile_pool(name="wpool", bufs=1) as wpool, \
         tc.tile_pool(name="tpool", bufs=2) as tpool, \
         tc.tile_pool(name="projpool", bufs=2) as projpool, \
         tc.tile_pool(name="ppool", bufs=1) as ppool, \
         tc.tile_pool(name="apool", bufs=2) as apool, \
         tc.tile_pool(name="statpool", bufs=16) as statpool, \
         tc.tile_pool(name="sps", bufs=4, space="PSUM") as sps, \
         tc.tile_pool(name="attps", bufs=2, space="PSUM") as attps, \
         tc.tile_pool(name="mmps", bufs=2, space="PSUM") as mmps:

        # ---------------- weights (one-time, one packed DMA) -------------
        walls = wpool.tile([128, 4, NT, 512], F16)  # w, i, n
        bq5_sb = wpool.tile([128, NT], F32)
        wq_sb = walls[:, 0, :, :]
        wk_sb = walls[:, 1, :, :]
        wv_sb = walls[:, 2, :, :]
        wo_sb = walls[:, 3, :, :]

        def load_weights(group):
            sl = slice(2 * group, 2 * group + 2)
            nc.gpsimd.dma_start(walls[:, sl, :, :], wall[:, sl, :, :])
            if group == 0:
                nc.gpsimd.dma_start(bq5_sb[:],
                                    bq5.rearrange("(i p) -> p i", p=128))

        # ---------------- per-frame state ----------------
        def alloc_state(f):
            st = {}
            # transposed: [d%128, stile, tensor, dblk, s%128] -- loaded
            # directly from the host-pre-transposed fp16 DRAM layout
            st["qkvT"] = tpool.tile([128, NT, 3, NT, 128], F16, tag="qkvT",
                                    name=f"qkvT_{f}")
            # projections: [dout%128 (head pair-stacked), pair, s]
            st["qT"] = projpool.tile([128, NT, 512], F16, tag="qT",
                                     name=f"qT_{f}")
            st["kT"] = projpool.tile([128, NT, 512], F16, tag="kT",
                                     name=f"kT_{f}")
            # V natural: [k%128, kblk, dout]
            st["vN"] = projpool.tile([128, NT, 512], F16, tag="vN", bufs=3,
                                     name=f"vN_{f}")
            return st

        def emit_load(f, st):
            nc.gpsimd.dma_start(st["qkvT"][:], qkv[f])

        def _rhs(st, t, i):
            # [din%128, (stile), s%128] strided view == [din, s] for chunk i
            return st["qkvT"][:, :, t, i, :]

        def emit_proj_qk_j(st, j):
            ps = mmps.tile([128, 512], F32, tag="mm", name=f"mmq_{j}")
            for i in range(NT):
                nc.tensor.matmul(
                    ps[:], wq_sb[:, i, 128 * j:128 * j + 128],
                    _rhs(st, 0, i), start=(i == 0), stop=(i == NT - 1))
            nc.scalar.activation(st["qT"][:, j, :], ps[:],
                                 AF.Identity, bias=bq5_sb[:, j:j + 1])
            ps = mmps.tile([128, 512], F32, tag="mm", name=f"mmk_{j}")
            for i in range(NT):
                nc.tensor.matmul(
                    ps[:], wk_sb[:, i, 128 * j:128 * j + 128],
                    _rhs(st, 1, i), start=(i == 0), stop=(i == NT - 1))
            nc.vector.tensor_copy(st["kT"][:, j, :], ps[:])

        def emit_proj_qk(st):
            for j in range(NT):
                emit_proj_qk_j(st, j)

        def emit_proj_v(st):
            for m in range(NT):
                ps = mmps.tile([128, 512], F32, tag="mm", name=f"mmv_{m}")
                for i in range(NT):
                    nc.tensor.matmul(
                        ps[:], st["qkvT"][:, m, 2, i, :],
                        wv_sb[:, i, :], start=(i == 0), stop=(i == NT - 1))
                if m % 2 == 0:
                    nc.scalar.activation(st["vN"][:, m, :], ps[:], AF.Copy)
                else:
                    nc.vector.tensor_copy(st["vN"][:, m, :], ps[:])

        # ---------------- attention ----------------
        def emit_scores(f, st, p2, si, a, norm_eng):
            m5 = statpool.tile([128, 2], F32, tag="m5",
                               name=f"m5_{f}_{si}_{a}")
            rs = statpool.tile([128, 2], F32, tag="rs",
                               name=f"rs_{f}_{si}_{a}")
            rsi = statpool.tile([128, 2], F32, tag="rsi",
                                name=f"rsi_{f}_{si}_{a}")
            for j, h in enumerate((2 * a, 2 * a + 1)):
                lo = 64 * (h % 2)
                s_ps = sps.tile([128, 512], F32, tag="s",
                                name=f"s_{f}_{si}_{h}")
                nc.tensor.matmul(
                    s_ps[:],
                    st["qT"][lo:lo + 64, a, 128 * si:128 * si + 128],
                    st["kT"][lo:lo + 64, a, :], start=True, stop=True)
                nc.vector.tensor_reduce(m5[:, j:j + 1], s_ps[:],
                                        mybir.AxisListType.X, ALU.min)
                nc.scalar.activation(p2[:, si % 2, h, :], s_ps[:],
                                     AF.Sigmoid, bias=m5[:, j:j + 1],
                                     scale=-1.0, accum_out=rs[:, j:j + 1])
            for j, h in enumerate((2 * a, 2 * a + 1)):
                nc.vector.reciprocal(rsi[:, j:j + 1], rs[:, j:j + 1])
                norm_eng.tensor_scalar(p2[:, si % 2, h, :],
                                       p2[:, si % 2, h, :],
                                       rsi[:, j:j + 1], None, ALU.mult)

        def emit_attend_half(f, st, attnT_h, aT, half):
            """All 4 head pairs for one q-half (256 cols); two pairs share
            one psum bank."""
            for a in range(4):
                a_ps = attps.tile([128, 256], F32, tag="att",
                                  name=f"aps_{f}_{half}_{a}")
                for h in (2 * a, 2 * a + 1):
                    lo = 64 * (h % 2)
                    for kb in range(NT):
                        nc.tensor.matmul(
                            a_ps[lo:lo + 64, :],
                            st["vN"][:, kb, 64 * h:64 * h + 64],
                            attnT_h[:, :, 4 * h + kb, :],
                            start=(kb == 0), stop=(kb == NT - 1),
                            tile_position=(0, lo))
                nc.vector.tensor_copy(
                    aT[:, a, 256 * half:256 * half + 256], a_ps[:])

        def emit_attend_quarter(f, st, attnT_h, aT, half, q2):
            for a in range(4):
                a_ps = attps.tile([128, 256], F32, tag="att",
                                  name=f"apsq_{f}_{half}_{q2}_{a}")
                for h in (2 * a, 2 * a + 1):
                    lo = 64 * (h % 2)
                    for kb in range(NT):
                        nc.tensor.matmul(
                            a_ps[lo:lo + 64, 0:128],
                            st["vN"][:, kb, 64 * h:64 * h + 64],
                            attnT_h[:, q2, 4 * h + kb, :],
                            start=(kb == 0), stop=(kb == NT - 1),
                            tile_position=(0, lo))
                nc.vector.tensor_copy(
                    aT[:, a, 256 * half + 128 * q2:
                       256 * half + 128 * q2 + 128], a_ps[:, 0:128])

        def emit_outproj(f, aT, outsb, stp):
            ps = mmps.tile([128, 512], F32, tag="mm", name=f"mmo_{f}_{stp}")
            for j in range(NT):
                nc.tensor.matmul(
                    ps[:], aT[:, j, 128 * stp:128 * stp + 128],
                    wo_sb[:, j, :], start=(j == 0), stop=(j == NT - 1))
            if stp % 2 == 0:
                nc.scalar.activation(outsb[:, stp, :], ps[:], AF.Copy)
            else:
                nc.vector.tensor_copy(outsb[:, stp, :], ps[:])
            if stp == NT - 1:
                nc.gpsimd.dma_start(
                    out[f].rearrange("(a p) d -> p a d", p=128), outsb[:])

        # ---------------- schedule ----------------
        st0 = alloc_state(0)
        load_weights(0)
        emit_load(0, st0)
        load_weights(1)
        # prime the sigmoid table set while DMAs run
        warm = wpool.tile([1, 2], F16)
        nc.vector.memset(warm[:], 0.0)
        nc.scalar.activation(warm[:], warm[:], AF.Sigmoid)
        # Q/K projection of frame 0 is interleaved per-j with si0's score
        # pairs inside the main loop (pair a only needs block j=a).

        st = st0
        prev = None  # (f, st, attnT_B, aT, outsb) awaiting half-B tail
        for f in range(FRAMES):
            nxt = alloc_state(f + 1) if f + 1 < FRAMES else None
            p2 = [ppool.tile([128, 2, H, 512], F16, tag=f"p{g}",
                             name=f"p_{f}_{g}") for g in range(2)]
            attnTs = [ppool.tile([128, 2, 32, 128], F16, tag=f"attnT{g}",
                                 name=f"attnT_{f}_{g}") for g in range(2)]
            aT = apool.tile([128, NT, 512], F16, tag="aT", name=f"aT_{f}")
            outsb = apool.tile([128, NT, 512], F16, tag="outsb",
                               name=f"outsb_{f}")
            fillers = {0: [], 1: [], 2: [], 3: []}
            if prev is not None:
                fp, stp, attnTsp, aTp, outsbp = prev
                fillers[0].append(
                    lambda: (emit_attend_half(fp, stp, attnTsp[0], aTp, 0),
                             emit_outproj(fp, aTp, outsbp, 0),
                             emit_outproj(fp, aTp, outsbp, 1)))
                fillers[1].append(
                    lambda: (emit_attend_half(fp, stp, attnTsp[1], aTp, 1),
                             emit_outproj(fp, aTp, outsbp, 2),
                             emit_outproj(fp, aTp, outsbp, 3)))
            if f == 0:
                fillers[0].append(lambda: emit_proj_v(st0))
            if nxt:
                fillers[1].append(lambda: emit_load(f + 1, nxt))
                fillers[2].append(lambda: emit_proj_qk(nxt))
                fillers[3].append(lambda: emit_proj_v(nxt))
            for si in range(NT):
                for a in range(4):
                    if f == 0 and si == 0:
                        emit_proj_qk_j(st, a)
                    on_dve = (si % 2 == 0 and a < 2)
                    norm_eng = nc.vector if on_dve else nc.gpsimd
                    emit_scores(f, st, p2[si // 2], si, a, norm_eng)
                if nxt is None:
                    # last frame: quarter transposes shorten the tail
                    nc.sync.dma_start(attnTs[si // 2][:, si % 2, :, :],
                                      p2[si // 2][:, si % 2, :, :],
                                      transpose=True)
                elif si % 2 == 1:
                    # transpose the completed half (si-1, si)
                    nc.sync.dma_start(attnTs[si // 2][:],
                                      p2[si // 2][:], transpose=True)
                for fn in fillers[si]:
                    fn()
                if nxt is None and si == 2:
                    emit_attend_half(f, st, attnTs[0], aT, 0)
                    emit_outproj(f, aT, outsb, 0)
                    emit_outproj(f, aT, outsb, 1)
                if nxt is None and si == 3:
                    emit_attend_quarter(f, st, attnTs[1], aT, 1, 0)
                    emit_outproj(f, aT, outsb, 2)
            if nxt is None:
                emit_attend_quarter(f, st, attnTs[1], aT, 1, 1)
                emit_outproj(f, aT, outsb, 3)
            prev = (f, st, attnTs, aT, outsb)
            st = nxt


def build_nc():
    nc = bacc.Bacc("TRN2", target_bir_lowering=False, debug=False,
                   num_devices=NCORES)
    qkv = nc.dram_tensor("qkv", (FRAMES, 128, NT, 3, NT, 128), F16,
                         kind="ExternalInput").ap()
    wall = nc.dram_tensor("wall", (128, 4, NT, D), F16,
                          kind="ExternalInput").ap()
    bq5 = nc.dram_tensor("bq5", (D,), F32, kind="ExternalInput").ap()
    out = nc.dram_tensor("out", (FRAMES, S, D), F16,
                         kind="ExternalOutput").ap()
    with tile.TileContext(nc) as tc:
        _emit(tc, nc, (qkv, wall, bq5, out))
    nc.compile()
    return nc


_NC = None


def _get_nc():
    global _NC
    if _NC is None:
        _NC = build_nc()
    return _NC


def make_in_maps(query_spikes, key_spikes, value_spikes, Wq, bq, Wk, bk,
                 Wv, bv, Wo, bo, modality_weights, temporal_sync,
                 query_modality, key_modality):
    qm = int(query_modality)
    km = int(key_modality)
    mw = np.asarray(modality_weights, np.float32)
    c = (mw[qm] * mw[km]) / np.float32(math.sqrt(HD))  # [H]
    scale_cols = np.repeat(-5.0 * c, HD).astype(np.float32)  # [D]
    wq_s = np.asarray(Wq, np.float32) * scale_cols[None, :]
    bq5 = (np.asarray(bq, np.float32) * scale_cols).astype(np.float32)

    f16 = lambda a: np.asarray(a, np.float32).astype(np.float16)
    # wall[p, w, i, n] = W_w[i*128+p, n]
    wall = np.stack([f16(wq_s), f16(Wk), f16(Wv), f16(Wo)])  # [4, D, D]
    wall = np.ascontiguousarray(
        wall.reshape(4, NT, 128, D).transpose(2, 0, 1, 3))
    shared = {"wall": wall, "bq5": bq5}
    # qkv_all[f, dp, st, t, db, sl] = tensor_t[f, st*128+sl, db*128+dp]
    qkv_all = np.stack([
        np.asarray(query_spikes, np.float32).reshape(B * T, S, D),
        np.asarray(key_spikes, np.float32).reshape(B * T, S, D),
        np.asarray(value_spikes, np.float32).reshape(B * T, S, D),
    ], axis=1).astype(np.float16)  # [B*T, 3, S, D]
    qkv_all = qkv_all.reshape(B * T, 3, NT, 128, NT, 128).transpose(
        0, 5, 2, 1, 4, 3)
    in_maps = []
    for core in range(NCORES):
        sl = slice(core * FRAMES, (core + 1) * FRAMES)
        in_maps.append({
            "qkv": np.ascontiguousarray(qkv_all[sl]),
            **shared,
        })
    return in_maps


def host_bias(Wv_np, bv_np, Wo_np, bo_np):
    return (np.asarray(bv_np, np.float64) @ np.asarray(Wo_np, np.float64)
            + np.asarray(bo_np, np.float64)).astype(np.float32)


def kernel(**inputs):
    nc = _get_nc()
    in_maps = make_in_maps(**inputs)
    res = bass_utils.run_bass_kernel_spmd(
        nc, in_maps, core_ids=list(range(NCORES)))
    out = np.concatenate([np.asarray(r["out"], np.float16).astype(np.float32)
                          for r in res.results], axis=0)
    out += host_bias(inputs["Wv"], inputs["bv"], inputs["Wo"], inputs["bo"])
    return out.reshape(B, T, S, D)
